# revision 9
# baseline (speedup 1.0000x reference)
"""Trainium2 Bass kernel for nn_Decoder_51144470561483 (GNN message passing).

Math (derived from the reference model):
  - The edge-encoder MLP(2->256->256->2)+LN2 output is affine in one scalar
    r = t/sqrt(t^2+eps), with t = h2e @ v3 + c0 (LN over 2 elements).
  - proc_edge first layer: x_dst rows are latlon zeros; x_src rows come from a
    5882-row table U = processor_features @ W1[:256]; the ea2 part is rank-1
    in r.  e = affine(r', r), so segment_sum reduces to per-node 7-group sums
    R = sum r_j and R' = sum r'_j.
  - proc_node first layer is rank-2 in (R', R); LN(256) is folded into the
    decoder's first layer via per-node scalars s1 = rstd, s2 = mu*rstd.

Sharding: edges and latlon nodes contiguous across 8 cores (the 7-edge groups
align with latlon nodes, so no collectives).  U[src] gathers are bf16 hi/lo
transpose-DMA-gathers (exact fp32 reconstruction).  Matmuls run in float32r.
"""

import sys

if "/opt/trn_rl_repo" not in sys.path:
    sys.path.insert(0, "/opt/trn_rl_repo")

import numpy as np

import concourse.bass as bass  # noqa: F401
import concourse.tile as tile
from concourse import mybir, bacc
from concourse.bass_utils import run_bass_kernel_spmd

# ---------------- problem constants (hardcoded) ----------------
NUM_H3 = 5882
NUM_LATLON = 64800
DEG = 7
NUM_EDGES = NUM_LATLON * DEG  # 453600
IN_DIM = 256
OUT_DIM = 78
EPS = 1e-5

NCORES = 8
EC = NUM_EDGES // NCORES  # 56700 edges per core
LC = NUM_LATLON // NCORES  # 8100 latlon nodes per core

CH = 448  # edge chunk (64 groups of 7)
NCH = 127  # chunks per core; 127*448 = 56896 >= 56700
EPAD = CH * NCH  # 56896
GB = 1792  # gather block (4 chunks), multiple of 128
NGB = 32  # gather blocks per core; 32*1792 = 57344 >= EPAD
NODC = 450  # node chunk
NNOD = 18  # node chunks; 18*450 = 8100
H3P = 5888  # padded U table rows (46*128)
NUT = H3P // 128  # 46

F32 = mybir.dt.float32
F32R = mybir.dt.float32r
BF16 = mybir.dt.bfloat16
I16 = mybir.dt.int16
AF = mybir.ActivationFunctionType
OP = mybir.AluOpType

_CACHE = {}


# ---------------- device program ----------------
def build_program():
    nc = bacc.Bacc("TRN2", target_bir_lowering=False, debug=False)

    def par(name, shape, dt):
        return nc.declare_dram_parameter(name, shape, dt, isOutput=False)

    # --- parameters (replicated weights; per-core data) ---
    ptT = par("ptT", [256, H3P], F32R)  # processor_features^T, padded
    w1a = par("w1a", [256, IN_DIM], F32R)  # proc_edge W1[:256]
    w1e = par("w1e", [2, 256], F32R)
    b1e = par("b1e", [128, 2], F32)
    w2e = par("w2e", [128, 2, 256], F32R)
    b2e = par("b2e", [128, 2], F32)
    v3 = par("v3", [128, 2, 1], F32R)
    pvec = par("pvec", [1, 256], F32R)
    qvec = par("qvec", [128, 2], F32)
    w2p = par("w2p", [128, 2, 256], F32R)
    b2p = par("b2p", [128, 2], F32)
    v2 = par("v2", [128, 2, 1], F32R)
    panb = par("panb", [2, 256], F32R)
    qn = par("qn", [128, 2], F32)
    w2n = par("w2n", [128, 2, 256], F32R)
    b2n = par("b2n", [128, 2], F32)
    w3n = par("w3n", [128, 2, 256], F32R)
    b3n = par("b3n", [128, 2], F32)
    gw = par("gw", [128, 2, 128], F32R)
    wbar = par("wbar", [128, 1], F32)
    cvec = par("cvec", [128, 1], F32)
    wd2 = par("wd2", [128, 128], F32R)
    bd2 = par("bd2", [128, 1], F32)
    wd3 = par("wd3", [128, OUT_DIM], F32R)
    ones21 = par("ones21", [128, 2, 1], F32R)
    onesrow = par("onesrow", [1, 128], F32R)
    cbias = par("cbias", [128, 3], F32)  # col0=c0, col1=c02, col2=EPS
    # per-core data
    ea_t = par("ea_t", [2, EPAD], F32R)  # [sin; cos] rows, padded
    idx16 = par("idx16", [128, NGB * (GB // 16)], I16)
    start_t = par("start_t", [OUT_DIM, LC], F32)  # start^T with bd3 folded in

    out_t = nc.declare_dram_parameter("out_t", [OUT_DIM, LC], F32, isOutput=True)

    # --- internal DRAM ---
    uhi_d = nc.dram_tensor("uhi_d", [H3P, IN_DIM], BF16)
    ulo_d = nc.dram_tensor("ulo_d", [H3P, IN_DIM], BF16)
    rf_d = nc.dram_tensor("rf_d", [EPAD], F32R)  # per-edge r (enc scalar)
    r_d = nc.dram_tensor("r_d", [128 * 64], F32R)  # R (7-group sums of r)
    r2_d = nc.dram_tensor("r2_d", [128 * 64], F32R)  # R' (sums of r')

    with tile.TileContext(nc) as tc:
        with (
            tc.tile_pool(name="consts", bufs=1) as cp,
            tc.tile_pool(name="rows", bufs=2) as rp,
            tc.tile_pool(name="big", bufs=1) as bigp,
            tc.tile_pool(name="ps2", bufs=2, space="PSUM") as ps2,
            tc.tile_pool(name="ps4", bufs=4, space="PSUM") as ps4,
        ):
            # ---- load constants ----
            def ctile(shape, dt, src):
                t = cp.tile(shape, dt, name=f"c_{src.name}")
                nc.sync.dma_start(t, src[:])
                return t

            w1e_sb = ctile([2, 256], F32R, w1e)
            b1e_sb = ctile([128, 2], F32, b1e)
            w2e_sb = ctile([128, 2, 256], F32R, w2e)
            b2e_sb = ctile([128, 2], F32, b2e)
            v3_sb = ctile([128, 2, 1], F32R, v3)
            pvec_sb = ctile([1, 256], F32R, pvec)
            qvec_sb = ctile([128, 2], F32, qvec)
            w2p_sb = ctile([128, 2, 256], F32R, w2p)
            b2p_sb = ctile([128, 2], F32, b2p)
            v2_sb = ctile([128, 2, 1], F32R, v2)
            panb_sb = ctile([2, 256], F32R, panb)
            qn_sb = ctile([128, 2], F32, qn)
            w2n_sb = ctile([128, 2, 256], F32R, w2n)
            b2n_sb = ctile([128, 2], F32, b2n)
            w3n_sb = ctile([128, 2, 256], F32R, w3n)
            b3n_sb = ctile([128, 2], F32, b3n)
            gw_sb = ctile([128, 2, 128], F32R, gw)
            wbar_sb = ctile([128, 1], F32, wbar)
            cvec_sb = ctile([128, 1], F32, cvec)
            wd2_sb = ctile([128, 128], F32R, wd2)
            bd2_sb = ctile([128, 1], F32, bd2)
            wd3_sb = ctile([128, OUT_DIM], F32R, wd3)
            ones21_sb = ctile([128, 2, 1], F32R, ones21)
            onesrow_sb = ctile([1, 128], F32R, onesrow)
            cbias_sb = ctile([128, 3], F32, cbias)
            idx_sb = cp.tile([128, NGB * (GB // 16)], I16, name="idx_sb")
            nc.sync.dma_start(idx_sb, idx16[:])

            # ---- U table build: U = P @ W1a (node-major tiles to DRAM) ----
            with tc.tile_pool(name="prol", bufs=2) as prp:
                with tc.tile_pool(name="prolc", bufs=1) as prc:
                    ptT_sb = prc.tile([128, 2, H3P], F32R, name="ptT_sb")
                    nc.sync.dma_start(
                        ptT_sb, ptT[:].rearrange("(ko ki) n -> ki ko n", ki=128)
                    )
                    w1a_sb = prc.tile([128, 2, IN_DIM], F32R, name="w1a_sb")
                    nc.sync.dma_start(
                        w1a_sb, w1a[:].rearrange("(ko ki) n -> ki ko n", ki=128)
                    )
                    for t in range(NUT):
                        psu = ps4.tile([128, IN_DIM], F32, name="psB")
                        for kc in range(2):
                            nc.tensor.matmul(
                                psu,
                                ptT_sb[:, kc, t * 128 : (t + 1) * 128],
                                w1a_sb[:, kc, :],
                                start=(kc == 0),
                                stop=(kc == 1),
                            )
                        uhi = prp.tile([128, IN_DIM], BF16, name="uhi")
                        nc.scalar.copy(uhi, psu)
                        ulo = prp.tile([128, IN_DIM], BF16, name="ulo")
                        nc.vector.tensor_tensor(
                            out=ulo, in0=psu, in1=uhi, op=OP.subtract
                        )
                        nc.sync.dma_start(uhi_d[t * 128 : (t + 1) * 128, :], uhi)
                        nc.sync.dma_start(ulo_d[t * 128 : (t + 1) * 128, :], ulo)

            # ---- helpers ----
            def mlp_layer(w_sb, x_sb, n):
                outs = []
                for mt in range(2):
                    pst = ps4.tile([128, n], F32, name="psB")
                    for kc in range(2):
                        nc.tensor.matmul(
                            pst,
                            w_sb[:, kc, mt * 128 : (mt + 1) * 128],
                            x_sb[:, kc, :],
                            start=(kc == 0),
                            stop=(kc == 1),
                        )
                    outs.append(pst)
                return outs

            def evict_relu(pool, psts, bias_sb, n, name):
                y = pool.tile([128, 2, n], F32R, name=name)
                for mt in range(2):
                    nc.scalar.activation(
                        out=y[:, mt, :],
                        in_=psts[mt],
                        func=AF.Relu,
                        bias=bias_sb[:, mt : mt + 1],
                    )
                return y

            def dot_row(v_sb, x_sb, n):
                pst = ps2.tile([1, n], F32, name="psT")
                for kc in range(2):
                    nc.tensor.matmul(
                        pst,
                        v_sb[:, kc, :],
                        x_sb[:, kc, :],
                        start=(kc == 0),
                        stop=(kc == 1),
                    )
                return pst

            # batch: r = (t+c0) * rsqrt((t+c0)^2 + eps); R = 7-group sums
            def batch_r(tb, c0_col, r_dram, rsum_dram, nm):
                tpc = bigp.tile([128, CH], F32, name=f"tpc_{nm}")
                nc.scalar.activation(
                    out=tpc, in_=tb, func=AF.Identity,
                    bias=cbias_sb[:, c0_col : c0_col + 1],
                )
                sq = bigp.tile([128, CH], F32, name=f"sq_{nm}")
                nc.scalar.activation(out=sq, in_=tpc, func=AF.Square)
                sd = bigp.tile([128, CH], F32, name=f"sd_{nm}")
                nc.scalar.activation(
                    out=sd, in_=sq, func=AF.Sqrt, bias=cbias_sb[:, 2:3]
                )
                rec = bigp.tile([128, CH], F32, name=f"rec_{nm}")
                nc.vector.reciprocal(rec, sd)
                rr = bigp.tile([128, CH], F32R, name=f"rr_{nm}")
                nc.vector.tensor_tensor(out=rr, in0=tpc, in1=rec, op=OP.mult)
                if r_dram is not None:
                    nc.sync.dma_start(
                        r_dram[:].rearrange("(p c) -> p c", p=NCH), rr[0:NCH, :]
                    )
                rs = bigp.tile([128, 64], F32R, name=f"rs_{nm}")
                with nc.allow_low_precision(reason="float32r is fp32-width"):
                    nc.vector.reduce_sum(
                        out=rs,
                        in_=rr[:].rearrange("p (g s) -> p g s", s=7),
                        axis=mybir.AxisListType.X,
                    )
                nc.sync.dma_start(rsum_dram[:].rearrange("(p g) -> p g", p=128), rs)

            # =================== phase E1 + E2 ===================
            epools = tc.tile_pool(name="scp", bufs=2)
            scp = epools.__enter__()
            gat_cm = tc.tile_pool(name="gat", bufs=2)
            gat = gat_cm.__enter__()
            eap_cm = tc.tile_pool(name="eact", bufs=2)
            eap = eap_cm.__enter__()
            tbuf = bigp.tile([128, CH], F32, name="tbuf")
            nc.vector.memset(tbuf, 0.0)

            sc_t = None
            for c in range(NCH):
                if c % 4 == 0:
                    blk = min(4, NCH - c)
                    sc_t = scp.tile([2, 4 * CH], F32R, name="sc_t")
                    nc.sync.dma_start(
                        sc_t[:, : blk * CH], ea_t[:, c * CH : (c + blk) * CH]
                    )
                sc = sc_t[:, (c % 4) * CH : (c % 4 + 1) * CH]
                psA = []
                for mt in range(2):
                    pst = ps2.tile([128, CH], F32, name="psA")
                    nc.tensor.matmul(
                        pst,
                        w1e_sb[:, mt * 128 : (mt + 1) * 128],
                        sc,
                        start=True,
                        stop=True,
                    )
                    psA.append(pst)
                h1e = evict_relu(eap, psA, b1e_sb, CH, "h1e")
                psB = mlp_layer(w2e_sb, h1e, CH)
                h2e = evict_relu(eap, psB, b2e_sb, CH, "h2e")
                psT = dot_row(v3_sb, h2e, CH)
                trow = rp.tile([1, CH], F32, name="trow")
                nc.scalar.copy(trow, psT)
                nc.sync.dma_start(tbuf[c : c + 1, :], trow)

            batch_r(tbuf, 0, rf_d, r_d, "enc")

            # =================== phase E2: proc_edge ===================
            t2buf = bigp.tile([128, CH], F32, name="t2buf")
            nc.vector.memset(t2buf, 0.0)

            ghi = glo = None
            for c in range(NCH):
                if c % 4 == 0:
                    b = c // 4
                    ghi = gat.tile([128, 2, GB], BF16, name="ghi")
                    nc.gpsimd.dma_gather(
                        out_ap=ghi[:],
                        in_ap=uhi_d[:],
                        idxs_ap=idx_sb[:, b * 112 : (b + 1) * 112],
                        num_idxs=GB,
                        num_idxs_reg=GB,
                        elem_size=IN_DIM,
                        transpose=True,
                        single_packet=False,
                    )
                    glo = gat.tile([128, 2, GB], BF16, name="glo")
                    nc.gpsimd.dma_gather(
                        out_ap=glo[:],
                        in_ap=ulo_d[:],
                        idxs_ap=idx_sb[:, b * 112 : (b + 1) * 112],
                        num_idxs=GB,
                        num_idxs_reg=GB,
                        elem_size=IN_DIM,
                        transpose=True,
                        single_packet=False,
                    )
                col = (c % 4) * CH
                rrow = rp.tile([1, CH], F32R, name="rrow")
                nc.sync.dma_start(rrow, rf_d[c * CH : (c + 1) * CH])
                psC = []
                for mt in range(2):
                    pst = ps2.tile([128, CH], F32, name="psA")
                    nc.tensor.matmul(
                        pst,
                        pvec_sb[:, mt * 128 : (mt + 1) * 128],
                        rrow,
                        start=True,
                        stop=True,
                    )
                    nc.vector.tensor_tensor(
                        out=pst, in0=pst, in1=ghi[:, mt, col : col + CH], op=OP.add
                    )
                    nc.vector.tensor_tensor(
                        out=pst, in0=pst, in1=glo[:, mt, col : col + CH], op=OP.add
                    )
                    psC.append(pst)
                h1 = evict_relu(eap, psC, qvec_sb, CH, "h1")
                psD = mlp_layer(w2p_sb, h1, CH)
                h2 = evict_relu(eap, psD, b2p_sb, CH, "h2")
                psT2 = dot_row(v2_sb, h2, CH)
                t2row = rp.tile([1, CH], F32, name="t2row")
                nc.scalar.copy(t2row, psT2)
                nc.sync.dma_start(t2buf[c : c + 1, :], t2row)

            batch_r(t2buf, 1, None, r2_d, "proc")
            eap_cm.__exit__(None, None, None)
            gat_cm.__exit__(None, None, None)
            epools.__exit__(None, None, None)

            # =================== node phase ===================
            nap_cm = tc.tile_pool(name="nact", bufs=2)
            nap = nap_cm.__enter__()
            for n in range(NNOD):
                o = n * NODC
                rr2 = rp.tile([2, NODC], F32R, name="rr2")
                nc.sync.dma_start(rr2[0:1, :], r2_d[o : o + NODC])  # R'
                nc.sync.dma_start(rr2[1:2, :], r_d[o : o + NODC])  # R
                psN = []
                for mt in range(2):
                    pst = ps4.tile([128, NODC], F32, name="psB")
                    nc.tensor.matmul(
                        pst,
                        panb_sb[:, mt * 128 : (mt + 1) * 128],
                        rr2[:],
                        start=True,
                        stop=True,
                    )
                    psN.append(pst)
                h1n = evict_relu(nap, psN, qn_sb, NODC, "h1n")
                psN2 = mlp_layer(w2n_sb, h1n, NODC)
                h2n = evict_relu(nap, psN2, b2n_sb, NODC, "h2n")
                psN3 = mlp_layer(w3n_sb, h2n, NODC)
                h = nap.tile([128, 2, NODC], F32R, name="hfin")
                hsq = nap.tile([128, 2, NODC], F32R, name="hsq")
                for mt in range(2):
                    nc.scalar.activation(
                        out=h[:, mt, :],
                        in_=psN3[mt],
                        func=AF.Identity,
                        bias=b3n_sb[:, mt : mt + 1],
                    )
                    nc.scalar.activation(
                        out=hsq[:, mt, :], in_=h[:, mt, :], func=AF.Square
                    )
                psMu = dot_row(ones21_sb, h, NODC)
                psM2 = dot_row(ones21_sb, hsq, NODC)
                mu = rp.tile([1, NODC], F32, name="mu")
                nc.scalar.activation(out=mu, in_=psMu, func=AF.Copy, scale=1.0 / 256.0)
                m2 = rp.tile([1, NODC], F32, name="m2")
                nc.scalar.activation(out=m2, in_=psM2, func=AF.Copy, scale=1.0 / 256.0)
                mu2 = rp.tile([1, NODC], F32, name="mu2")
                nc.vector.tensor_tensor(out=mu2, in0=mu, in1=mu, op=OP.mult)
                nc.vector.tensor_tensor(out=mu2, in0=m2, in1=mu2, op=OP.subtract)
                sdn = rp.tile([1, NODC], F32, name="sdn")
                nc.scalar.activation(out=sdn, in_=mu2, func=AF.Sqrt, bias=cbias_sb[0:1, 2:3])
                s1 = rp.tile([1, NODC], F32R, name="s1")
                with nc.allow_low_precision(reason="float32r is fp32-width"):
                    nc.vector.reciprocal(s1, sdn)
                s2 = rp.tile([1, NODC], F32R, name="s2")
                nc.vector.tensor_tensor(out=s2, in0=mu, in1=s1, op=OP.mult)
                psS1 = ps2.tile([128, NODC], F32, name="psA")
                nc.tensor.matmul(psS1, onesrow_sb[:], s1, start=True, stop=True)
                psS2 = ps2.tile([128, NODC], F32, name="psA")
                nc.tensor.matmul(psS2, onesrow_sb[:], s2, start=True, stop=True)
                psE = ps4.tile([128, NODC], F32, name="psB")
                for kc in range(2):
                    nc.tensor.matmul(
                        psE,
                        gw_sb[:, kc, :],
                        h[:, kc, :],
                        start=(kc == 0),
                        stop=(kc == 1),
                    )
                bc1 = nap.tile([128, NODC], F32, name="bc1")
                nc.scalar.copy(bc1, psS1)
                e1 = nap.tile([128, NODC], F32, name="e1")
                nc.vector.tensor_tensor(out=e1, in0=psE, in1=bc1, op=OP.mult)
                e2 = nap.tile([128, NODC], F32, name="e2")
                nc.vector.tensor_scalar_mul(e2, psS2, wbar_sb[:, 0:1])
                nc.vector.tensor_tensor(out=e1, in0=e1, in1=e2, op=OP.subtract)
                hd1 = nap.tile([128, NODC], F32R, name="hd1")
                nc.scalar.activation(
                    out=hd1, in_=e1, func=AF.Relu, bias=cvec_sb[:, 0:1]
                )
                psF = ps4.tile([128, NODC], F32, name="psB")
                nc.tensor.matmul(psF, wd2_sb[:], hd1, start=True, stop=True)
                hd2 = nap.tile([128, NODC], F32R, name="hd2")
                nc.scalar.activation(
                    out=hd2, in_=psF, func=AF.Relu, bias=bd2_sb[:, 0:1]
                )
                psG = ps4.tile([OUT_DIM, NODC], F32, name="psB")
                nc.tensor.matmul(psG, wd3_sb[:], hd2, start=True, stop=True)
                stt = nap.tile([OUT_DIM, NODC], F32, name="stt")
                nc.sync.dma_start(stt, start_t[:, o : o + NODC])
                nc.vector.tensor_tensor(out=stt, in0=psG, in1=stt, op=OP.add)
                nc.sync.dma_start(out_t[:, o : o + NODC], stt)
            nap_cm.__exit__(None, None, None)

    nc.finalize()
    return nc


# ---------------- host side ----------------
def _wrap_idx(src_pad):
    """int16 gather index layout: [128, NGB*112], wrapped by 16, replicated."""
    out = np.zeros((16, NGB * (GB // 16)), np.int16)
    for b in range(NGB):
        blk = src_pad[b * GB : (b + 1) * GB]
        w = blk.reshape(GB // 16, 16).T  # [16, 112]
        out[:, b * 112 : (b + 1) * 112] = w
    return np.tile(out, (8, 1))


def _np_reference(processor_features, start_features, latlon_nodes, edge_index,
                  edge_attr, edge_enc, proc_edge, proc_node, dec):
    """Pure numpy fallback replicating the jax reference (slow, exact)."""

    def mlp(params, x, norm):
        n_lin = len(params) - (2 if norm else 0)
        for i in range(0, n_lin, 2):
            x = x @ params[i] + params[i + 1]
            if i + 2 < n_lin:
                x = np.maximum(x, 0.0)
        if norm:
            g, beta = params[-2], params[-1]
            mu = x.mean(-1, keepdims=True)
            var = x.var(-1, keepdims=True)
            x = (x - mu) / np.sqrt(var + EPS) * g + beta
        return x

    ea = mlp(edge_enc, edge_attr, True)
    x = np.concatenate([processor_features, latlon_nodes], axis=0)
    src, dst = edge_index[0], edge_index[1]
    e_in = np.concatenate([x[src], x[dst], ea], axis=-1)
    e = mlp(proc_edge, e_in, True) + ea
    agg = np.zeros((NUM_H3 + NUM_LATLON, 2), np.float32)
    np.add.at(agg, dst, e)
    n_in = np.concatenate([x, agg], axis=-1)
    x = mlp(proc_node, n_in, True) + x
    out = x[NUM_H3:]
    out = mlp(dec, out, False) + start_features
    return out.astype(np.float32)


def _prep(processor_features, start_features, edge_index, edge_attr,
          edge_enc, proc_edge, proc_node, dec):
    """Build replicated weight arrays and the per-core input maps."""
    f = np.float32
    W1e, b1e, W2e, b2e, W3e, b3e, g_e, be_e = edge_enc
    W1, b1, W2, b2, W3, b3, g_p, be_p = proc_edge
    W1n, b1n, W2n, b2n, W3n, b3n, g_n, be_n = proc_node
    Wd1, bd1, Wd2, bd2, Wd3, bd3 = dec

    v3 = (W3e[:, 0] - W3e[:, 1]) / 2.0
    c0 = float((b3e[0] - b3e[1]) / 2.0)
    p_vec = g_e[0] * W1[512] - g_e[1] * W1[513]
    q_vec = be_e[0] * W1[512] + be_e[1] * W1[513] + b1
    v2 = (W3[:, 0] - W3[:, 1]) / 2.0
    c02 = float((b3[0] - b3[1]) / 2.0)
    wa0, wa1 = W1n[256], W1n[257]
    pa = g_p[0] * wa0 - g_p[1] * wa1
    pb = g_e[0] * wa0 - g_e[1] * wa1
    qn_vec = 7.0 * (be_p[0] + be_e[0]) * wa0 + 7.0 * (be_p[1] + be_e[1]) * wa1 + b1n
    GW = g_n[:, None] * Wd1
    wbar = Wd1.T @ g_n
    cvec = Wd1.T @ be_n + bd1

    def kpart(vec):  # [256] -> [128, 2]
        return np.ascontiguousarray(vec.reshape(2, 128).T)

    def kmaj(mat, ncol):  # [256, ncol] -> [128, 2, ncol]
        return np.ascontiguousarray(mat.reshape(2, 128, ncol).transpose(1, 0, 2))

    ptT = np.zeros((256, H3P), f)
    ptT[:, :NUM_H3] = processor_features.T

    shared = dict(
        ptT=ptT,
        w1a=W1[:256],
        w1e=W1e,
        b1e=kpart(b1e),
        w2e=kmaj(W2e, 256),
        b2e=kpart(b2e),
        v3=kmaj(v3[:, None], 1),
        pvec=p_vec[None, :],
        qvec=kpart(q_vec),
        w2p=kmaj(W2, 256),
        b2p=kpart(b2),
        v2=kmaj(v2[:, None], 1),
        panb=np.stack([pa, pb]),
        qn=kpart(qn_vec),
        w2n=kmaj(W2n, 256),
        b2n=kpart(b2n),
        w3n=kmaj(W3n, 256),
        b3n=kpart(b3n),
        gw=kmaj(GW, 128),
        wbar=wbar[:, None],
        cvec=cvec[:, None],
        wd2=Wd2,
        bd2=bd2[:, None],
        wd3=Wd3,
        ones21=np.ones((128, 2, 1), f),
        onesrow=np.ones((1, 128), f),
        cbias=np.tile(np.array([[c0, c02, EPS]], f), (128, 1)),
    )
    shared = {k: np.ascontiguousarray(v, f) for k, v in shared.items()}

    src = np.asarray(edge_index[0])
    in_maps = []
    for c in range(NCORES):
        e0 = c * EC
        src_pad = np.zeros(NGB * GB, np.int16)
        src_pad[:EC] = src[e0 : e0 + EC].astype(np.int16)
        ea_pad = np.zeros((2, EPAD), f)
        ea_pad[:, :EC] = edge_attr[e0 : e0 + EC].T
        st = start_features[c * LC : (c + 1) * LC] + bd3[None, :]
        m = dict(shared)
        m["ea_t"] = ea_pad
        m["idx16"] = _wrap_idx(src_pad)
        m["start_t"] = np.ascontiguousarray(st.T, f)
        in_maps.append(m)
    return in_maps, c0, c02


def kernel(**inputs):
    processor_features = np.asarray(inputs["processor_features"], np.float32)
    start_features = np.asarray(inputs["start_features"], np.float32)
    latlon_nodes = np.asarray(inputs["latlon_nodes"], np.float32)
    edge_index = np.asarray(inputs["edge_index"], np.int32)
    edge_attr = np.asarray(inputs["edge_attr"], np.float32)
    edge_enc = [np.asarray(a, np.float32) for a in inputs["edge_enc"]]
    proc_edge = [np.asarray(a, np.float32) for a in inputs["proc_edge"]]
    proc_node = [np.asarray(a, np.float32) for a in inputs["proc_node"]]
    dec = [np.asarray(a, np.float32) for a in inputs["dec"]]

    # structural checks gating the fast path
    expect_tgt = NUM_H3 + np.repeat(np.arange(NUM_LATLON, dtype=np.int64), DEG)
    ok = (
        edge_index.shape == (2, NUM_EDGES)
        and np.array_equal(edge_index[1], expect_tgt)
        and edge_index[0].min() >= 0
        and edge_index[0].max() < NUM_H3
        and not latlon_nodes.any()
    )
    if not ok:
        return _np_reference(
            processor_features, start_features, latlon_nodes, edge_index,
            edge_attr, edge_enc, proc_edge, proc_node, dec,
        )

    in_maps, c0, c02 = _prep(
        processor_features, start_features, edge_index, edge_attr,
        edge_enc, proc_edge, proc_node, dec,
    )

    if "prog" not in _CACHE:
        _CACHE["prog"] = build_program()
    nc = _CACHE["prog"]

    res = run_bass_kernel_spmd(nc, in_maps, core_ids=list(range(NCORES)))
    outs = [r["out_t"].T for r in res.results]
    return np.ascontiguousarray(np.concatenate(outs, axis=0), np.float32)
